# revision 24
# baseline (speedup 1.0000x reference)
"""Trainium2 Bass kernel for the 2-layer GAT node-classification head.

The reference reads only h2[mask_idx] and x[mask_idx] for the classifier, so
the exact computation collapses to mask_idx's 2-hop in-neighborhood:

  layer 1: h1 = x @ W1 is needed only at sources of in-edges of V1
           (V1 = sources of mask's in-edges), one row per edge in S2.
  layer 2: h2 = elu(gat1) @ W2 is needed only at rows V1, and the final
           classifier (fc -> cls, two consecutive affine maps) folds into a
           single [1536, 2] matrix on the host, so layer-2's GEMM contracts
           into W2 @ fold (4 columns: 2 logits + a_src2 + a_dst2).

Sharding over 8 cores:
  - layer-1 GEMM + attention by head (H1=8 -> head i on core i)
  - layer-2 folded GEMM by contraction block (core i contracts the head-i
    block of elu(h1)); one AllReduce(add) of the small partial
  - everything after the AllReduce is tiny and runs redundantly on all cores

Host preprocessing: gather + transpose the needed x rows (index-select is
part of sharding), fold attention vectors and the classifier into the weight
matrices, and build one-hot scatter matrices plus a uniform-stride edge
layout so segment softmax lowers to batched strided reductions.
"""

import numpy as np

import concourse.bass as bass
import concourse.mybir as mybir
import concourse.tile as tile
from concourse import bacc
from concourse.bass_utils import run_bass_kernel_spmd
from concourse.masks import make_identity

NCORES = 8
P = 128
C = 768          # input feature dim
H1 = 8           # layer-1 heads
OUT = 768        # per-head feature dim
KC = C // P      # 6 k-chunks of 128 over a 768 contraction
W2F = 4          # folded layer-2 rhs cols: [cls0 cls1 a_src2 a_dst2]
NEG = -1.0e30    # padding logit

f32 = mybir.dt.float32
f32r = mybir.dt.float32r
bf16 = mybir.dt.bfloat16
i32 = mybir.dt.int32
GEMM_DT = f32r   # single-pass fp32 matmul: full DMA bytes, 4x PE rate


# ---------------------------------------------------------------- host graph
def _preprocess(edge_index, mask_idx, n_nodes):
    """Extract the 2-hop in-neighborhood of mask_idx and pack it into
    uniform-stride group tiles. Everything in meta is compile-time python."""
    ei = np.asarray(edge_index).astype(np.int64)
    m = int(np.asarray(mask_idx))
    src_all = np.concatenate([ei[0], np.arange(n_nodes, dtype=np.int64)])
    dst_all = np.concatenate([ei[1], np.arange(n_nodes, dtype=np.int64)])

    s1_pos = np.nonzero(dst_all == m)[0]          # in-edges of m (incl self-loop)
    s1_src = src_all[s1_pos].tolist()
    v1 = list(dict.fromkeys(s1_src))              # unique sources, first-occurrence
    v1n = len(v1)
    v1p = max(v1n, 2)
    assert v1n <= P, f"in-degree of mask node too large: {v1n}"
    v1_row = {v: r for r, v in enumerate(v1)}
    s1n = len(s1_src)
    n_s1t = max(1, -(-s1n // P))
    s1p = n_s1t * P
    assert s1p <= 512, f"mask in-degree {s1n} exceeds 512"
    # layer-2 gather is the identity when every in-edge has a distinct source
    s1_ident = s1n == v1n

    # S2: in-edges of each v in V1, at uniform stride gmax within tiles
    groups = [src_all[np.nonzero(dst_all == v)[0]].tolist() for v in v1]
    gmax = max(len(g) for g in groups)
    assert gmax <= P, f"in-degree {gmax} exceeds {P}"
    gpt = P // gmax                               # groups per 128-slot tile
    n_et = -(-v1n // gpt)
    s2p = n_et * P

    src_ids = np.zeros(s2p, np.int64)             # padded with node 0
    m01 = np.zeros((s2p, v1p), np.float32)
    padbias = np.full((H1, s2p), NEG, np.float32)
    ngs = []                                      # groups in each tile
    for t in range(n_et):
        gs = groups[t * gpt:(t + 1) * gpt]
        ngs.append(len(gs))
        for j, srcs in enumerate(gs):
            v_row = t * gpt + j
            lo = t * P + j * gmax
            src_ids[lo:lo + len(srcs)] = srcs
            m01[lo:lo + len(srcs), v_row] = 1.0
            padbias[:, lo:lo + len(srcs)] = 0.0

    v1_ids = np.zeros(v1p, np.int64)
    v1_ids[:v1n] = np.array(v1, np.int64)

    g_mat = np.zeros((v1p, s1p), np.float32)      # a_src2 gather (src of S1 edge)
    gm_mat = np.zeros((v1p, s1p), np.float32)     # a_dst2 broadcast (row of m)
    for e, s in enumerate(s1_src):
        g_mat[v1_row[s], e] = 1.0
        gm_mat[v1_row[m], e] = 1.0
    gt_mat = np.ascontiguousarray(g_mat.T)        # [s1p, v1p]

    meta = dict(m=m, v1n=v1n, v1p=v1p, s1n=s1n, s1p=s1p, n_s1t=n_s1t,
                n_et=n_et, gmax=gmax, ngs=tuple(ngs), s1_ident=s1_ident)
    host = dict(src_ids=src_ids, v1_ids=v1_ids, m01=m01, padbias=padbias,
                m01t=np.ascontiguousarray(m01.T), g=g_mat, gm=gm_mat,
                gt=gt_mat)
    return meta, host


def _chunked(w):
    """[K, N] -> [128, (K//128)*N] with chunk-major free layout for one DMA."""
    k, n = w.shape
    assert k % P == 0
    return np.ascontiguousarray(
        w.reshape(k // P, P, n).transpose(1, 0, 2).reshape(P, (k // P) * n))


def _colmajor(v):
    """[768] -> [128, 6] column-chunk layout."""
    return np.ascontiguousarray(v.reshape(KC, P).T)


def _const_layout(meta):
    """Column layout of the packed-constants tensor, shared host/build."""
    v1p, s1p, n_s1t = meta["v1p"], meta["s1p"], meta["n_s1t"]
    s2p = meta["n_et"] * P
    pieces = [
        ("xvt", P, KC * v1p),
        ("ones", NCORES, 1),
        ("wd1", P, KC * H1),
        ("w2f", P, KC * W2F),
        ("wfb", P, KC * 2),
        ("b1", P, KC),
        ("xm", P, KC),
        ("m01", P, meta["n_et"] * v1p),
        ("m01t", v1p, s2p),
        ("g", v1p, s1p),
        ("gm", v1p, s1p),
        ("gt", P, n_s1t * v1p),
        ("padbias", H1, s2p),
        ("bias3", 1, 2),
        ("head", H1, 1),
    ]
    lay, off = {}, 0
    for name, rows, cols in pieces:
        lay[name] = (rows, off, cols)
        off += cols
    return lay, off


# ---------------------------------------------------------------- bass build
def _build(meta):
    v1p, s1p, n_s1t, n_et = meta["v1p"], meta["s1p"], meta["n_s1t"], meta["n_et"]
    gmax, ngs = meta["gmax"], meta["ngs"]
    s2p = n_et * P
    packed = n_s1t == 1
    ccw = P * 3 if packed else 2 * v1p + s1p      # AllGather payload
    lay, cw = _const_layout(meta)

    nc = bacc.Bacc("TRN2", target_bir_lowering=False, debug=False,
                   enable_asserts=True, num_devices=NCORES)

    d_xga = nc.dram_tensor("xga", [P, KC * (s2p + H1)], GEMM_DT,
                           kind="ExternalInput")
    d_cst = nc.dram_tensor("cst", [P, cw], f32, kind="ExternalInput")
    d_w1 = [nc.dram_tensor(f"w1c{c}", [P, OUT], GEMM_DT, kind="ExternalInput")
            for c in range(KC)]
    d_res = nc.dram_tensor("res", [1, 2], f32, kind="ExternalOutput")

    with tile.TileContext(nc) as tc:
        with (
            tc.tile_pool(name="const", bufs=1) as cpool,
            tc.tile_pool(name="sbuf", bufs=2) as sb,
            tc.tile_pool(name="big", bufs=1) as bigp,
            tc.tile_pool(name="ps", bufs=1, space="PSUM") as ps,
            tc.tile_pool(name="dram", bufs=1, space="DRAM") as dr,
        ):
            # ---- input loads: critical pieces first ----
            xga_sb = bigp.tile([P, KC, s2p + H1], GEMM_DT, tag="xga")
            nc.sync.dma_start(out=xga_sb[:], in_=d_xga[:].rearrange(
                "p (k n) -> p k n", k=KC))
            cst = cpool.tile([P, cw], f32, tag="cst")
            nc.sync.dma_start(out=cst[:], in_=d_cst[:])
            w1_sb = [bigp.tile([P, OUT], GEMM_DT, tag=f"w1_{c}", name=f"w1_{c}")
                     for c in range(KC)]
            for c in range(KC):
                nc.sync.dma_start(out=w1_sb[c][:], in_=d_w1[c][:])

            def cv(name):
                rows, off, cols = lay[name]
                return cst[0:rows, off:off + cols]

            xvt_v = cv("xvt").rearrange("p (k n) -> p k n", k=KC)
            ones_v = cv("ones")
            wd1_v = cv("wd1").rearrange("p (k n) -> p k n", k=KC)
            w2f_v = cv("w2f").rearrange("p (k n) -> p k n", k=KC)
            wfb_v = cv("wfb").rearrange("p (k n) -> p k n", k=KC)
            b1_v = cv("b1")
            xm_v = cv("xm")
            m01_v = cv("m01").rearrange("p (t n) -> p t n", t=n_et)
            m01t_v = cv("m01t")
            g_v = cv("g")
            gm_v = cv("gm")
            gt_v = cv("gt").rearrange("p (k n) -> p k n", k=n_s1t)
            pad_v = cv("padbias")
            bias3_v = cv("bias3")
            head_v = cv("head")

            ident = cpool.tile([P, P], f32, tag="ident")
            make_identity(nc, ident[:])

            # ---- attention inputs: a_src per edge, a_dst per node ----
            asT_sb = []
            for t in range(n_et):
                ap_s = ps.tile([P, H1], f32, tag="mm_b", name="ap_s")
                for c in range(KC):
                    nc.tensor.matmul(out=ap_s[:],
                                     lhsT=xga_sb[:, c, t * P:(t + 1) * P],
                                     rhs=xga_sb[:, c, s2p:s2p + H1],
                                     start=(c == 0), stop=(c == KC - 1))
                asb = sb.tile([P, H1], f32, tag=f"as_{t}", name=f"as_{t}")
                nc.vector.tensor_copy(out=asb[:], in_=ap_s[:])
                at = ps.tile([H1, P], f32, tag="tp", bufs=2, name="at")
                nc.tensor.transpose(out=at[:], in_=asb[:], identity=ident[:])
                at2 = sb.tile([H1, P], f32, tag=f"asT_{t}", name=f"asT_{t}")
                nc.vector.tensor_copy(out=at2[:], in_=at[:])
                asT_sb.append(at2)
            adv_ps = ps.tile([v1p, H1], f32, tag="mm_b", name="adv")
            for c in range(KC):
                nc.tensor.matmul(out=adv_ps[:], lhsT=xvt_v[:, c, :],
                                 rhs=wd1_v[:, c, :],
                                 start=(c == 0), stop=(c == KC - 1))
            adv_sb = sb.tile([v1p, H1], f32, tag="adv_sb")
            nc.vector.tensor_copy(out=adv_sb[:], in_=adv_ps[:])

            # ---- layer-1 logits + batched segment softmax (all heads) ----
            logit = sb.tile([H1, s2p], f32, tag="logit")
            for t in range(n_et):
                adT = ps.tile([H1, P], f32, tag="tp", bufs=2, name="adT")
                nc.tensor.matmul(out=adT[:], lhsT=adv_sb[:],
                                 rhs=m01t_v[:, t * P:(t + 1) * P],
                                 start=True, stop=True)
                nc.vector.tensor_add(out=logit[:, t * P:(t + 1) * P],
                                     in0=asT_sb[t][:], in1=adT[:])
            # leaky relu + padding mask
            tmp = sb.tile([H1, s2p], f32, tag="ltmp")
            nc.vector.tensor_scalar_mul(out=tmp[:], in0=logit[:], scalar1=0.2)
            nc.vector.tensor_tensor(out=logit[:], in0=logit[:], in1=tmp[:],
                                    op=mybir.AluOpType.max)
            nc.vector.tensor_add(out=logit[:], in0=logit[:], in1=pad_v)
            # per-group max-shift, exp, normalize (strided batched form)
            for t in range(n_et):
                ng = ngs[t]
                view = logit[:, t * P:t * P + ng * gmax].rearrange(
                    "h (g e) -> h g e", e=gmax)
                mx = sb.tile([H1, ng], f32, tag=f"mx{t}", name=f"mx{t}")
                nc.vector.reduce_max(out=mx[:], in_=view,
                                     axis=mybir.AxisListType.X)
                mxb = mx[:].rearrange("h (g o) -> h g o", o=1).to_broadcast(
                    [H1, ng, gmax])
                nc.vector.tensor_tensor(out=view, in0=view, in1=mxb,
                                        op=mybir.AluOpType.subtract)
            nc.scalar.activation(out=logit[:], in_=logit[:],
                                 func=mybir.ActivationFunctionType.Exp)
            for t in range(n_et):
                ng = ngs[t]
                view = logit[:, t * P:t * P + ng * gmax].rearrange(
                    "h (g e) -> h g e", e=gmax)
                sm = sb.tile([H1, ng], f32, tag=f"sm{t}", name=f"sm{t}")
                nc.vector.reduce_sum(out=sm[:], in_=view,
                                     axis=mybir.AxisListType.X)
                rc = sb.tile([H1, ng], f32, tag=f"rc{t}", name=f"rc{t}")
                nc.vector.reciprocal(out=rc[:], in_=sm[:])
                rcb = rc[:].rearrange("h (g o) -> h g o", o=1).to_broadcast(
                    [H1, ng, gmax])
                nc.vector.tensor_tensor(out=view, in0=view, in1=rcb,
                                        op=mybir.AluOpType.mult)
            # alpha column for this core's head + alpha-scaled selection
            a_sel = []
            for t in range(n_et):
                acol = ps.tile([P, 1], f32, tag="tp", bufs=2, name="acol")
                nc.tensor.matmul(out=acol[:],
                                 lhsT=logit[:, t * P:(t + 1) * P],
                                 rhs=head_v, start=True, stop=True)
                acs = sb.tile([P, 1], f32, tag=f"acol_sb{t}", name=f"acol_sb{t}")
                nc.vector.tensor_copy(out=acs[:], in_=acol[:])
                asel = sb.tile([P, v1p], f32, tag=f"a_sel{t}", name=f"a_sel{t}")
                nc.vector.tensor_scalar(out=asel[:], in0=m01_v[:, t, :],
                                        scalar1=acs[:], scalar2=None,
                                        op0=mybir.AluOpType.mult)
                a_sel.append(asel)

            # ---- the big per-head GEMM1: h1 = x_src @ W1_head ----
            h1_sb = []
            for t in range(n_et):
                hp_a = ps.tile([P, 512], f32, tag="mm_a", name="hp_a")
                hp_b = ps.tile([P, 256], f32, tag="mm_b", name="hp_b")
                for c in range(KC):
                    nc.tensor.matmul(out=hp_a[:],
                                     lhsT=xga_sb[:, c, t * P:(t + 1) * P],
                                     rhs=w1_sb[c][:, 0:512],
                                     start=(c == 0), stop=(c == KC - 1))
                for c in range(KC):
                    nc.tensor.matmul(out=hp_b[:],
                                     lhsT=xga_sb[:, c, t * P:(t + 1) * P],
                                     rhs=w1_sb[c][:, 512:OUT],
                                     start=(c == 0), stop=(c == KC - 1))
                h1t = sb.tile([P, OUT], f32, tag=f"h1_{t}", name=f"h1_{t}")
                nc.vector.tensor_copy(out=h1t[:, 0:512], in_=hp_a[:])
                nc.vector.tensor_copy(out=h1t[:, 512:OUT], in_=hp_b[:])
                h1_sb.append(h1t)

            # ---- xm @ Wf_bot partial (independent of the collective) ----
            oxm_ps = ps.tile([1, 2], f32, tag="oxm", name="oxm_ps")
            for c in range(KC):
                nc.tensor.matmul(out=oxm_ps[:], lhsT=xm_v[:, c:c + 1],
                                 rhs=wfb_v[:, c, :],
                                 start=(c == 0), stop=(c == KC - 1))
            oxm_sb = sb.tile([1, 2], f32, tag="oxm_sb")
            nc.vector.tensor_copy(out=oxm_sb[:], in_=oxm_ps[:])

            # ---- aggregation + bias, batched elu, folded layer-2 partial ----
            helu = sb.tile([P, KC, v1p], f32, tag="helu")
            assert KC * v1p <= 512
            agg = ps.tile([P, KC * v1p], f32, tag="agg", bufs=2, name="agg")
            for c in range(KC):
                for t in range(n_et):
                    nc.tensor.matmul(out=agg[:, c * v1p:(c + 1) * v1p],
                                     lhsT=h1_sb[t][:, c * P:(c + 1) * P],
                                     rhs=a_sel[t][:], start=(t == 0),
                                     stop=(t == n_et - 1))
            b1b = b1_v.rearrange("p (k o) -> p k o", o=1).to_broadcast(
                [P, KC, v1p])
            nc.vector.tensor_tensor(
                out=helu[:], in0=agg[:].rearrange("p (k n) -> p k n", k=KC),
                in1=b1b, op=mybir.AluOpType.add)
            # elu(x) = max(x,0) + exp(min(x,0)) - 1, one pass over all chunks
            hall = helu[:].rearrange("p k n -> p (k n)")
            mn = sb.tile([P, KC * v1p], f32, tag="mn")
            nc.vector.tensor_scalar_min(out=mn[:], in0=hall, scalar1=0.0)
            nc.scalar.activation(out=mn[:], in_=mn[:],
                                 func=mybir.ActivationFunctionType.Exp)
            nc.vector.tensor_scalar_max(out=hall, in0=hall, scalar1=0.0)
            nc.vector.tensor_add(out=hall, in0=hall, in1=mn[:])
            nc.vector.tensor_scalar_add(out=hall, in0=hall, scalar1=-1.0)
            h2f_ps = ps.tile([v1p, W2F], f32, tag="h2f", name="h2f")
            for c in range(KC):
                nc.tensor.matmul(out=h2f_ps[:], lhsT=helu[:, c, :],
                                 rhs=w2f_v[:, c, :],
                                 start=(c == 0), stop=(c == KC - 1))
            h2f_part = sb.tile([v1p, W2F], f32, tag="h2f_part")
            nc.vector.tensor_copy(out=h2f_part[:], in_=h2f_ps[:])

            # layer-2 logits are linear in h2f -> fold into the AllGather
            cc_in = dr.tile([1, ccw], f32, tag="cc_in", name="cc_in")
            cc_out = dr.tile([1, NCORES * ccw], f32, tag="cc_out",
                             name="cc_out")
            if packed:
                # transposed logits land partition-major next to h2f cols so
                # one staging tile covers the whole payload in a single DMA
                lgT_ps = ps.tile([P, 1], f32, tag="tp", bufs=2, name="lgT")
                nc.tensor.matmul(out=lgT_ps[:], lhsT=g_v[:, 0:P],
                                 rhs=h2f_part[:, 2:3], start=True, stop=False)
                nc.tensor.matmul(out=lgT_ps[:], lhsT=gm_v[:, 0:P],
                                 rhs=h2f_part[:, 3:4], start=False, stop=True)
                stg = sb.tile([P, 3], f32, tag="stg")
                nc.vector.tensor_copy(out=stg[:, 0:1], in_=lgT_ps[:])
                nc.vector.tensor_copy(out=stg[0:v1p, 1:3], in_=h2f_ps[:, 0:2])
                nc.gpsimd.dma_start(
                    out=cc_in[0:1, :].rearrange("a (p w) -> (a p) w", p=P),
                    in_=stg[:])
            else:
                lg2_ps = ps.tile([1, s1p], f32, tag="mm_a", name="lg2")
                nc.tensor.matmul(out=lg2_ps[:], lhsT=h2f_part[:, 2:3],
                                 rhs=g_v, start=True, stop=False)
                nc.tensor.matmul(out=lg2_ps[:], lhsT=h2f_part[:, 3:4],
                                 rhs=gm_v, start=False, stop=True)
                lg2_sb = sb.tile([1, s1p], f32, tag="lg2_sb")
                nc.vector.tensor_copy(out=lg2_sb[:], in_=lg2_ps[:])
                nc.sync.dma_start(
                    out=cc_in[0:1, 0:2 * v1p].rearrange("a (v f) -> (a v) f",
                                                        v=v1p),
                    in_=h2f_part[:, 0:2])
                nc.sync.dma_start(out=cc_in[0:1, 2 * v1p:ccw], in_=lg2_sb[:])
            nc.gpsimd.collective_compute(
                "AllGather", mybir.AluOpType.bypass,
                replica_groups=[list(range(NCORES))],
                ins=[cc_in.opt()], outs=[cc_out.opt()])
            ccg8 = sb.tile([NCORES, ccw], f32, tag="ccg8")
            nc.gpsimd.dma_start(
                out=ccg8[:],
                in_=cc_out[0:1, :].rearrange("a (r w) -> (a r) w", r=NCORES))
            red_ps = ps.tile([1, ccw], f32, tag="mm_a", name="red_ps")
            nc.tensor.matmul(out=red_ps[:], lhsT=ones_v, rhs=ccg8[:],
                             start=True, stop=True)

            # ---- layer-2 softmax at mask node (redundant on all cores) ----
            s1n, v1n = meta["s1n"], meta["v1n"]
            if packed:
                raw2 = red_ps[:].rearrange("a (p w) -> a w p", w=3)[:, 0, :]
                h2view_src = red_ps[:].rearrange(
                    "a (p w) -> a w p", w=3)[:, 1:3, 0:v1n]
            else:
                raw2 = red_ps[:, 2 * v1p:ccw]
                h2view_src = red_ps[:, 0:2 * v1p].rearrange(
                    "a (v f) -> a f v", f=2)[:, :, 0:v1n]
            al2w = P if packed else s1p
            al2t = sb.tile([1, al2w], f32, tag="al2t")
            tmp2 = sb.tile([1, al2w], f32, tag="tmp2")
            nc.vector.tensor_scalar_mul(out=tmp2[:], in0=raw2, scalar1=0.2)
            nc.vector.tensor_tensor(out=al2t[:], in0=raw2, in1=tmp2[:],
                                    op=mybir.AluOpType.max)
            al2 = al2t[:]
            nmx2 = sb.tile([1, 1], f32, tag="nmx2")
            nc.vector.reduce_max(out=nmx2[:], in_=al2[:, 0:s1n],
                                 axis=mybir.AxisListType.X, negate=True)
            nc.scalar.activation(out=al2[:, 0:s1n], in_=al2[:, 0:s1n],
                                 func=mybir.ActivationFunctionType.Exp,
                                 bias=nmx2[:, 0:1])
            sm2 = sb.tile([1, 1], f32, tag="sm2")
            nc.vector.reduce_sum(out=sm2[:], in_=al2[:, 0:s1n],
                                 axis=mybir.AxisListType.X)

            res_sb = sb.tile([1, 2], f32, tag="res_sb")
            if meta["s1_ident"]:
                # sources unique -> alpha2 aligns with V1 rows directly
                wb = al2[:, 0:v1n].rearrange(
                    "a (o v) -> a o v", o=1).to_broadcast([1, 2, v1n])
                prod = sb.tile([1, 2, v1n], f32, tag="prod")
                nc.vector.tensor_tensor(out=prod[:], in0=wb, in1=h2view_src,
                                        op=mybir.AluOpType.mult)
                nc.vector.reduce_sum(out=res_sb[:], in_=prod[:],
                                     axis=mybir.AxisListType.X)
                # normalize by the softmax denominator
                rc2 = sb.tile([1, 1], f32, tag="rc2")
                nc.vector.reciprocal(out=rc2[:], in_=sm2[:])
                nc.vector.tensor_scalar_mul(out=res_sb[:], in0=res_sb[:],
                                            scalar1=rc2[:])
            else:
                # general path: w = (GT @ alpha2^T) / denom, out = w.T @ h2f
                w_ps = ps.tile([1, v1p], f32, tag="mm_b", name="w_ps")
                for k in range(n_s1t):
                    a2T = ps.tile([P, 1], f32, tag="tp", bufs=2, name="a2T")
                    nc.tensor.transpose(out=a2T[:],
                                        in_=al2[:, k * P:(k + 1) * P],
                                        identity=ident[:1, :1])
                    a2Ts = sb.tile([P, 1], f32, tag="a2Ts")
                    nc.vector.tensor_copy(out=a2Ts[:], in_=a2T[:])
                    nc.tensor.matmul(out=w_ps[:], lhsT=a2Ts[:],
                                     rhs=gt_v[:, k, :],
                                     start=(k == 0), stop=(k == n_s1t - 1))
                rc2 = sb.tile([1, 1], f32, tag="rc2")
                nc.vector.reciprocal(out=rc2[:], in_=sm2[:])
                w_row = sb.tile([1, v1p], f32, tag="w_row")
                nc.vector.tensor_scalar_mul(out=w_row[:], in0=w_ps[:],
                                            scalar1=rc2[:])
                wb = w_row[:, 0:v1n].rearrange(
                    "a (o v) -> a o v", o=1).to_broadcast([1, 2, v1n])
                prod2 = sb.tile([1, 2, v1n], f32, tag="prod2")
                nc.vector.tensor_tensor(out=prod2[:], in0=wb, in1=h2view_src,
                                        op=mybir.AluOpType.mult)
                nc.vector.reduce_sum(out=res_sb[:], in_=prod2[:],
                                     axis=mybir.AxisListType.X)

            nc.vector.tensor_add(out=res_sb[:], in0=res_sb[:], in1=oxm_sb[:])
            nc.vector.tensor_add(out=res_sb[:], in0=res_sb[:], in1=bias3_v)
            nc.sync.dma_start(out=d_res[:], in_=res_sb[:])

    nc.compile()
    return nc


_CACHE = {}


def _get_nc(meta):
    key = repr(sorted(meta.items()))
    if key not in _CACHE:
        _CACHE[key] = _build(meta)
    return _CACHE[key]


def make_in_maps(**inputs):
    """Host preprocessing: shard/fold inputs into per-core input maps."""
    x = np.asarray(inputs["x"], np.float32)
    n_nodes = x.shape[0]
    meta, host = _preprocess(inputs["edge_index"], inputs["mask_idx"], n_nodes)

    W1 = np.asarray(inputs["W1"], np.float32)
    att_s1 = np.asarray(inputs["att_src1"], np.float32)
    att_d1 = np.asarray(inputs["att_dst1"], np.float32)
    b1 = np.asarray(inputs["b1"], np.float32)
    W2 = np.asarray(inputs["W2"], np.float32)
    att_s2 = np.asarray(inputs["att_src2"], np.float32)
    att_d2 = np.asarray(inputs["att_dst2"], np.float32)
    b2 = np.asarray(inputs["b2"], np.float32)
    fc_w = np.asarray(inputs["fc_w"], np.float32)
    fc_b = np.asarray(inputs["fc_b"], np.float32)
    cls_w = np.asarray(inputs["cls_w"], np.float32)
    cls_b = np.asarray(inputs["cls_b"], np.float32)

    Ws1 = np.einsum("chf,hf->ch", W1.reshape(C, H1, OUT), att_s1)  # [C, H1]
    Wd1 = np.einsum("chf,hf->ch", W1.reshape(C, H1, OUT), att_d1)
    Ws2 = W2 @ att_s2[0]                                           # [H1*OUT]
    Wd2 = W2 @ att_d2[0]
    # classifier fold: out = cat @ fc_w @ cls_w + (fc_b @ cls_w + cls_b)
    wf = fc_w @ cls_w                                              # [1536, 2]
    wf_top, wf_bot = wf[:OUT], wf[OUT:]
    w2fold = W2 @ wf_top                                           # [6144, 2]
    bias3 = (b2 @ wf_top + fc_b @ cls_w + cls_b).reshape(1, 2).astype(np.float32)

    n_s1t, v1p, s1p = meta["n_s1t"], meta["v1p"], meta["s1p"]
    n_et = meta["n_et"]
    s2p = n_et * P
    gt_pad = np.zeros((n_s1t * P, v1p), np.float32)
    gt_pad[:s1p] = host["gt"]

    # pre-gathered + pre-transposed x rows (index-select = sharding)
    s2p_ = meta["n_et"] * P
    xg = x[host["src_ids"]]                                        # [s2p, 768]
    xgt3 = np.ascontiguousarray(xg.T).reshape(KC, P, s2p_)
    ws13 = Ws1.reshape(KC, P, H1)
    xga = np.concatenate([xgt3, ws13], axis=2)                     # [KC,128,s2p+8]
    xga = np.ascontiguousarray(
        xga.transpose(1, 0, 2).reshape(P, KC * (s2p_ + H1)))
    xv = x[host["v1_ids"]]                                         # [v1p, 768]
    xvt = _chunked(np.ascontiguousarray(xv.T))                     # [128, KC*v1p]
    ones8 = np.ones((NCORES, 1), np.float32)

    lay, cw = _const_layout(meta)

    def fill(cst, name, arr):
        rows, off, cols = lay[name]
        assert arr.shape == (rows, cols), (name, arr.shape, (rows, cols))
        cst[0:rows, off:off + cols] = arr

    m01_pack = np.concatenate(
        [host["m01"][t * P:(t + 1) * P] for t in range(n_et)], axis=1)

    in_maps = []
    for i in range(NCORES):
        w1blk = np.ascontiguousarray(W1[:, i * OUT:(i + 1) * OUT])
        w2fblk = np.concatenate(
            [w2fold[i * OUT:(i + 1) * OUT, :],
             Ws2[i * OUT:(i + 1) * OUT, None],
             Wd2[i * OUT:(i + 1) * OUT, None]], axis=1)            # [768, 4]
        head = np.zeros((H1, 1), np.float32)
        head[i % H1, 0] = 1.0
        cst = np.zeros((P, cw), np.float32)
        fill(cst, "xvt", xvt)
        fill(cst, "ones", ones8)
        fill(cst, "wd1", _chunked(Wd1))
        fill(cst, "w2f", _chunked(w2fblk))
        fill(cst, "wfb", _chunked(np.ascontiguousarray(wf_bot)))
        fill(cst, "b1", _colmajor(b1[i * OUT:(i + 1) * OUT]))
        fill(cst, "xm", _colmajor(np.ascontiguousarray(x[meta["m"]])))
        fill(cst, "m01", m01_pack)
        fill(cst, "m01t", host["m01t"])
        fill(cst, "g", host["g"])
        fill(cst, "gm", host["gm"])
        fill(cst, "gt", _chunked(gt_pad))
        fill(cst, "padbias", host["padbias"])
        fill(cst, "bias3", bias3)
        fill(cst, "head", head)
        im = {
            "xga": xga,
            "cst": cst,
        }
        for c in range(KC):
            im[f"w1c{c}"] = np.ascontiguousarray(w1blk[c * P:(c + 1) * P, :])
        in_maps.append(im)
    return meta, in_maps


def kernel(**inputs):
    meta, in_maps = make_in_maps(**inputs)
    nc = _get_nc(meta)
    res = run_bass_kernel_spmd(nc, in_maps, core_ids=list(range(NCORES)))
    return res.results[0]["res"].astype(np.float32)


# revision 25
# speedup vs baseline: 1.0215x; 1.0215x over previous
"""Trainium2 Bass kernel for the 2-layer GAT node-classification head.

The reference reads only h2[mask_idx] and x[mask_idx] for the classifier, so
the exact computation collapses to mask_idx's 2-hop in-neighborhood:

  layer 1: h1 = x @ W1 is needed only at sources of in-edges of V1
           (V1 = sources of mask's in-edges), one row per edge in S2.
  layer 2: h2 = elu(gat1) @ W2 is needed only at rows V1, and the final
           classifier (fc -> cls, two consecutive affine maps) folds into a
           single [1536, 2] matrix on the host, so layer-2's GEMM contracts
           into W2 @ fold (4 columns: 2 logits + a_src2 + a_dst2).

Sharding over 8 cores:
  - layer-1 GEMM + attention by head (H1=8 -> head i on core i)
  - layer-2 folded GEMM by contraction block (core i contracts the head-i
    block of elu(h1)); one AllReduce(add) of the small partial
  - everything after the AllReduce is tiny and runs redundantly on all cores

Host preprocessing: gather + transpose the needed x rows (index-select is
part of sharding), fold attention vectors and the classifier into the weight
matrices, and build one-hot scatter matrices plus a uniform-stride edge
layout so segment softmax lowers to batched strided reductions.
"""

import numpy as np

import concourse.bass as bass
import concourse.mybir as mybir
import concourse.tile as tile
from concourse import bacc
from concourse.bass_utils import run_bass_kernel_spmd
from concourse.masks import make_identity

NCORES = 8
P = 128
C = 768          # input feature dim
H1 = 8           # layer-1 heads
OUT = 768        # per-head feature dim
KC = C // P      # 6 k-chunks of 128 over a 768 contraction
W2F = 4          # folded layer-2 rhs cols: [cls0 cls1 a_src2 a_dst2]
NEG = -1.0e30    # padding logit

f32 = mybir.dt.float32
f32r = mybir.dt.float32r
bf16 = mybir.dt.bfloat16
i32 = mybir.dt.int32
GEMM_DT = f32r   # single-pass fp32 matmul: full DMA bytes, 4x PE rate


# ---------------------------------------------------------------- host graph
def _preprocess(edge_index, mask_idx, n_nodes):
    """Extract the 2-hop in-neighborhood of mask_idx and pack it into
    uniform-stride group tiles. Everything in meta is compile-time python."""
    ei = np.asarray(edge_index).astype(np.int64)
    m = int(np.asarray(mask_idx))
    src_all = np.concatenate([ei[0], np.arange(n_nodes, dtype=np.int64)])
    dst_all = np.concatenate([ei[1], np.arange(n_nodes, dtype=np.int64)])

    s1_pos = np.nonzero(dst_all == m)[0]          # in-edges of m (incl self-loop)
    s1_src = src_all[s1_pos].tolist()
    v1 = list(dict.fromkeys(s1_src))              # unique sources, first-occurrence
    v1n = len(v1)
    v1p = max(v1n, 2)
    assert v1n <= P, f"in-degree of mask node too large: {v1n}"
    v1_row = {v: r for r, v in enumerate(v1)}
    s1n = len(s1_src)
    n_s1t = max(1, -(-s1n // P))
    s1p = n_s1t * P
    assert s1p <= 512, f"mask in-degree {s1n} exceeds 512"
    # layer-2 gather is the identity when every in-edge has a distinct source
    s1_ident = s1n == v1n

    # S2: in-edges of each v in V1, at uniform stride gmax within tiles
    groups = [src_all[np.nonzero(dst_all == v)[0]].tolist() for v in v1]
    gmax = max(len(g) for g in groups)
    assert gmax <= P, f"in-degree {gmax} exceeds {P}"
    gpt = P // gmax                               # groups per 128-slot tile
    n_et = -(-v1n // gpt)
    s2p = n_et * P

    src_ids = np.zeros(s2p, np.int64)             # padded with node 0
    m01 = np.zeros((s2p, v1p), np.float32)
    padbias = np.full((H1, s2p), NEG, np.float32)
    ngs = []                                      # groups in each tile
    for t in range(n_et):
        gs = groups[t * gpt:(t + 1) * gpt]
        ngs.append(len(gs))
        for j, srcs in enumerate(gs):
            v_row = t * gpt + j
            lo = t * P + j * gmax
            src_ids[lo:lo + len(srcs)] = srcs
            m01[lo:lo + len(srcs), v_row] = 1.0
            padbias[:, lo:lo + len(srcs)] = 0.0

    v1_ids = np.zeros(v1p, np.int64)
    v1_ids[:v1n] = np.array(v1, np.int64)

    g_mat = np.zeros((v1p, s1p), np.float32)      # a_src2 gather (src of S1 edge)
    gm_mat = np.zeros((v1p, s1p), np.float32)     # a_dst2 broadcast (row of m)
    for e, s in enumerate(s1_src):
        g_mat[v1_row[s], e] = 1.0
        gm_mat[v1_row[m], e] = 1.0
    gt_mat = np.ascontiguousarray(g_mat.T)        # [s1p, v1p]

    meta = dict(m=m, v1n=v1n, v1p=v1p, s1n=s1n, s1p=s1p, n_s1t=n_s1t,
                n_et=n_et, gmax=gmax, ngs=tuple(ngs), s1_ident=s1_ident)
    host = dict(src_ids=src_ids, v1_ids=v1_ids, m01=m01, padbias=padbias,
                m01t=np.ascontiguousarray(m01.T), g=g_mat, gm=gm_mat,
                gt=gt_mat)
    return meta, host


def _chunked(w):
    """[K, N] -> [128, (K//128)*N] with chunk-major free layout for one DMA."""
    k, n = w.shape
    assert k % P == 0
    return np.ascontiguousarray(
        w.reshape(k // P, P, n).transpose(1, 0, 2).reshape(P, (k // P) * n))


def _colmajor(v):
    """[768] -> [128, 6] column-chunk layout."""
    return np.ascontiguousarray(v.reshape(KC, P).T)


def _const_layout(meta):
    """Column layout of the packed-constants tensor, shared host/build."""
    v1p, s1p, n_s1t = meta["v1p"], meta["s1p"], meta["n_s1t"]
    s2p = meta["n_et"] * P
    pieces = [
        ("xvt", P, KC * v1p),
        ("ones", NCORES, 1),
        ("wd1", P, KC * H1),
        ("w2f", P, KC * W2F),
        ("wfb", P, KC * 2),
        ("b1", P, KC),
        ("xm", P, KC),
        ("m01", P, meta["n_et"] * v1p),
        ("m01t", v1p, s2p),
        ("g", v1p, s1p),
        ("gm", v1p, s1p),
        ("gt", P, n_s1t * v1p),
        ("padbias", H1, s2p),
        ("bias3", 1, 2),
        ("head", H1, 1),
    ]
    lay, off = {}, 0
    for name, rows, cols in pieces:
        lay[name] = (rows, off, cols)
        off += cols
    return lay, off


# ---------------------------------------------------------------- bass build
def _build(meta):
    v1p, s1p, n_s1t, n_et = meta["v1p"], meta["s1p"], meta["n_s1t"], meta["n_et"]
    gmax, ngs = meta["gmax"], meta["ngs"]
    s2p = n_et * P
    packed = n_s1t == 1
    ccw = P * 3 if packed else 2 * v1p + s1p      # AllGather payload
    lay, cw = _const_layout(meta)

    nc = bacc.Bacc("TRN2", target_bir_lowering=False, debug=False,
                   enable_asserts=True, num_devices=NCORES)

    d_xga = nc.dram_tensor("xga", [P, KC * (s2p + H1)], GEMM_DT,
                           kind="ExternalInput")
    d_cst = nc.dram_tensor("cst", [P, cw], f32, kind="ExternalInput")
    d_w1 = [nc.dram_tensor(f"w1c{c}", [P, OUT], GEMM_DT, kind="ExternalInput")
            for c in range(KC)]
    d_res = nc.dram_tensor("res", [1, 2], f32, kind="ExternalOutput")

    with tile.TileContext(nc) as tc:
        with (
            tc.tile_pool(name="const", bufs=1) as cpool,
            tc.tile_pool(name="sbuf", bufs=2) as sb,
            tc.tile_pool(name="big", bufs=1) as bigp,
            tc.tile_pool(name="ps", bufs=1, space="PSUM") as ps,
            tc.tile_pool(name="dram", bufs=1, space="DRAM") as dr,
        ):
            # ---- input loads: critical pieces first ----
            xga_sb = bigp.tile([P, KC, s2p + H1], GEMM_DT, tag="xga")
            nc.sync.dma_start(out=xga_sb[:], in_=d_xga[:].rearrange(
                "p (k n) -> p k n", k=KC))
            cst = cpool.tile([P, cw], f32, tag="cst")
            nc.sync.dma_start(out=cst[:], in_=d_cst[:])
            w1_sb = [bigp.tile([P, OUT], GEMM_DT, tag=f"w1_{c}", name=f"w1_{c}")
                     for c in range(KC)]
            for c in range(KC):
                nc.sync.dma_start(out=w1_sb[c][:], in_=d_w1[c][:])

            def cv(name):
                rows, off, cols = lay[name]
                return cst[0:rows, off:off + cols]

            xvt_v = cv("xvt").rearrange("p (k n) -> p k n", k=KC)
            ones_v = cv("ones")
            wd1_v = cv("wd1").rearrange("p (k n) -> p k n", k=KC)
            w2f_v = cv("w2f").rearrange("p (k n) -> p k n", k=KC)
            wfb_v = cv("wfb").rearrange("p (k n) -> p k n", k=KC)
            b1_v = cv("b1")
            xm_v = cv("xm")
            m01_v = cv("m01").rearrange("p (t n) -> p t n", t=n_et)
            m01t_v = cv("m01t")
            g_v = cv("g")
            gm_v = cv("gm")
            gt_v = cv("gt").rearrange("p (k n) -> p k n", k=n_s1t)
            pad_v = cv("padbias")
            bias3_v = cv("bias3")
            head_v = cv("head")

            ident = cpool.tile([P, P], f32, tag="ident")
            make_identity(nc, ident[:])

            # ---- attention inputs: a_src per edge, a_dst per node ----
            asT_sb = []
            for t in range(n_et):
                ap_s = ps.tile([P, H1], f32, tag="mm_b", name="ap_s")
                for c in range(KC):
                    nc.tensor.matmul(out=ap_s[:],
                                     lhsT=xga_sb[:, c, t * P:(t + 1) * P],
                                     rhs=xga_sb[:, c, s2p:s2p + H1],
                                     start=(c == 0), stop=(c == KC - 1))
                asb = sb.tile([P, H1], f32, tag=f"as_{t}", name=f"as_{t}")
                nc.vector.tensor_copy(out=asb[:], in_=ap_s[:])
                at = ps.tile([H1, P], f32, tag="tp", bufs=2, name="at")
                nc.tensor.transpose(out=at[:], in_=asb[:], identity=ident[:])
                at2 = sb.tile([H1, P], f32, tag=f"asT_{t}", name=f"asT_{t}")
                nc.vector.tensor_copy(out=at2[:], in_=at[:])
                asT_sb.append(at2)
            adv_ps = ps.tile([v1p, H1], f32, tag="mm_b", name="adv")
            for c in range(KC):
                nc.tensor.matmul(out=adv_ps[:], lhsT=xvt_v[:, c, :],
                                 rhs=wd1_v[:, c, :],
                                 start=(c == 0), stop=(c == KC - 1))
            adv_sb = sb.tile([v1p, H1], f32, tag="adv_sb")
            nc.vector.tensor_copy(out=adv_sb[:], in_=adv_ps[:])

            # ---- layer-1 logits + batched segment softmax (all heads) ----
            logit = sb.tile([H1, s2p], f32, tag="logit")
            for t in range(n_et):
                adT = ps.tile([H1, P], f32, tag="tp", bufs=2, name="adT")
                nc.tensor.matmul(out=adT[:], lhsT=adv_sb[:],
                                 rhs=m01t_v[:, t * P:(t + 1) * P],
                                 start=True, stop=True)
                nc.vector.tensor_add(out=logit[:, t * P:(t + 1) * P],
                                     in0=asT_sb[t][:], in1=adT[:])
            # leaky relu + padding mask
            tmp = sb.tile([H1, s2p], f32, tag="ltmp")
            nc.vector.tensor_scalar_mul(out=tmp[:], in0=logit[:], scalar1=0.2)
            nc.vector.tensor_tensor(out=logit[:], in0=logit[:], in1=tmp[:],
                                    op=mybir.AluOpType.max)
            nc.vector.tensor_add(out=logit[:], in0=logit[:], in1=pad_v)
            # per-group max-shift, exp, normalize (strided batched form)
            for t in range(n_et):
                ng = ngs[t]
                view = logit[:, t * P:t * P + ng * gmax].rearrange(
                    "h (g e) -> h g e", e=gmax)
                mx = sb.tile([H1, ng], f32, tag=f"mx{t}", name=f"mx{t}")
                nc.vector.reduce_max(out=mx[:], in_=view,
                                     axis=mybir.AxisListType.X)
                mxb = mx[:].rearrange("h (g o) -> h g o", o=1).to_broadcast(
                    [H1, ng, gmax])
                nc.vector.tensor_tensor(out=view, in0=view, in1=mxb,
                                        op=mybir.AluOpType.subtract)
            nc.scalar.activation(out=logit[:], in_=logit[:],
                                 func=mybir.ActivationFunctionType.Exp)
            for t in range(n_et):
                ng = ngs[t]
                view = logit[:, t * P:t * P + ng * gmax].rearrange(
                    "h (g e) -> h g e", e=gmax)
                sm = sb.tile([H1, ng], f32, tag=f"sm{t}", name=f"sm{t}")
                nc.vector.reduce_sum(out=sm[:], in_=view,
                                     axis=mybir.AxisListType.X)
                rc = sb.tile([H1, ng], f32, tag=f"rc{t}", name=f"rc{t}")
                nc.vector.reciprocal(out=rc[:], in_=sm[:])
                rcb = rc[:].rearrange("h (g o) -> h g o", o=1).to_broadcast(
                    [H1, ng, gmax])
                nc.vector.tensor_tensor(out=view, in0=view, in1=rcb,
                                        op=mybir.AluOpType.mult)
            # alpha column for this core's head + alpha-scaled selection
            a_sel = []
            for t in range(n_et):
                acol = ps.tile([P, 1], f32, tag="tp", bufs=2, name="acol")
                nc.tensor.matmul(out=acol[:],
                                 lhsT=logit[:, t * P:(t + 1) * P],
                                 rhs=head_v, start=True, stop=True)
                acs = sb.tile([P, 1], f32, tag=f"acol_sb{t}", name=f"acol_sb{t}")
                nc.vector.tensor_copy(out=acs[:], in_=acol[:])
                asel = sb.tile([P, v1p], f32, tag=f"a_sel{t}", name=f"a_sel{t}")
                nc.vector.tensor_scalar(out=asel[:], in0=m01_v[:, t, :],
                                        scalar1=acs[:], scalar2=None,
                                        op0=mybir.AluOpType.mult)
                a_sel.append(asel)

            # ---- the big per-head GEMM1: h1 = x_src @ W1_head ----
            h1_sb = []
            for t in range(n_et):
                hp_a = ps.tile([P, 512], f32, tag="mm_a", name="hp_a")
                hp_b = ps.tile([P, 256], f32, tag="mm_b", name="hp_b")
                for c in range(KC):
                    nc.tensor.matmul(out=hp_a[:],
                                     lhsT=xga_sb[:, c, t * P:(t + 1) * P],
                                     rhs=w1_sb[c][:, 0:512],
                                     start=(c == 0), stop=(c == KC - 1))
                for c in range(KC):
                    nc.tensor.matmul(out=hp_b[:],
                                     lhsT=xga_sb[:, c, t * P:(t + 1) * P],
                                     rhs=w1_sb[c][:, 512:OUT],
                                     start=(c == 0), stop=(c == KC - 1))
                h1t = sb.tile([P, OUT], f32, tag=f"h1_{t}", name=f"h1_{t}")
                nc.vector.tensor_copy(out=h1t[:, 0:512], in_=hp_a[:])
                nc.vector.tensor_copy(out=h1t[:, 512:OUT], in_=hp_b[:])
                h1_sb.append(h1t)

            # ---- xm @ Wf_bot partial (independent of the collective) ----
            oxm_ps = ps.tile([1, 2], f32, tag="oxm", name="oxm_ps")
            for c in range(KC):
                nc.tensor.matmul(out=oxm_ps[:], lhsT=xm_v[:, c:c + 1],
                                 rhs=wfb_v[:, c, :],
                                 start=(c == 0), stop=(c == KC - 1))
            oxm_sb = sb.tile([1, 2], f32, tag="oxm_sb")
            nc.vector.tensor_copy(out=oxm_sb[:], in_=oxm_ps[:])

            # ---- aggregation + bias, batched elu, folded layer-2 partial ----
            helu = sb.tile([P, KC, v1p], f32, tag="helu")
            assert KC * v1p <= 512
            agg = ps.tile([P, KC * v1p], f32, tag="agg", bufs=2, name="agg")
            for c in range(KC):
                for t in range(n_et):
                    nc.tensor.matmul(out=agg[:, c * v1p:(c + 1) * v1p],
                                     lhsT=h1_sb[t][:, c * P:(c + 1) * P],
                                     rhs=a_sel[t][:], start=(t == 0),
                                     stop=(t == n_et - 1))
            b1b = b1_v.rearrange("p (k o) -> p k o", o=1).to_broadcast(
                [P, KC, v1p])
            nc.vector.tensor_tensor(
                out=helu[:], in0=agg[:].rearrange("p (k n) -> p k n", k=KC),
                in1=b1b, op=mybir.AluOpType.add)
            # elu(x) = max(x,0) + exp(min(x,0)) - 1, one pass over all chunks
            hall = helu[:].rearrange("p k n -> p (k n)")
            mn = sb.tile([P, KC * v1p], f32, tag="mn")
            nc.vector.tensor_scalar_min(out=mn[:], in0=hall, scalar1=0.0)
            nc.scalar.activation(out=mn[:], in_=mn[:],
                                 func=mybir.ActivationFunctionType.Exp)
            nc.vector.tensor_scalar_max(out=hall, in0=hall, scalar1=0.0)
            nc.vector.tensor_add(out=hall, in0=hall, in1=mn[:])
            nc.vector.tensor_scalar_add(out=hall, in0=hall, scalar1=-1.0)
            h2f_ps = ps.tile([v1p, W2F], f32, tag="h2f", name="h2f")
            for c in range(KC):
                nc.tensor.matmul(out=h2f_ps[:], lhsT=helu[:, c, :],
                                 rhs=w2f_v[:, c, :],
                                 start=(c == 0), stop=(c == KC - 1))
            h2f_part = sb.tile([v1p, W2F], f32, tag="h2f_part")
            nc.vector.tensor_copy(out=h2f_part[:], in_=h2f_ps[:])

            # layer-2 logits are linear in h2f -> fold into the AllGather
            cc_in = dr.tile([1, ccw], f32, tag="cc_in", name="cc_in")
            cc_out = dr.tile([1, NCORES * ccw], f32, tag="cc_out",
                             name="cc_out")
            if packed:
                # transposed logits land partition-major next to h2f cols so
                # one staging tile covers the whole payload in a single DMA
                lgT_ps = ps.tile([P, 1], f32, tag="tp", bufs=2, name="lgT")
                nc.tensor.matmul(out=lgT_ps[:], lhsT=g_v[:, 0:P],
                                 rhs=h2f_part[:, 2:3], start=True, stop=False)
                nc.tensor.matmul(out=lgT_ps[:], lhsT=gm_v[:, 0:P],
                                 rhs=h2f_part[:, 3:4], start=False, stop=True)
                stg = sb.tile([P, 3], f32, tag="stg")
                nc.vector.tensor_copy(out=stg[:, 0:1], in_=lgT_ps[:])
                nc.vector.tensor_copy(out=stg[0:v1p, 1:3], in_=h2f_ps[:, 0:2])
                nc.sync.dma_start(
                    out=cc_in[0:1, :].rearrange("a (p w) -> (a p) w", p=P),
                    in_=stg[:])
            else:
                lg2_ps = ps.tile([1, s1p], f32, tag="mm_a", name="lg2")
                nc.tensor.matmul(out=lg2_ps[:], lhsT=h2f_part[:, 2:3],
                                 rhs=g_v, start=True, stop=False)
                nc.tensor.matmul(out=lg2_ps[:], lhsT=h2f_part[:, 3:4],
                                 rhs=gm_v, start=False, stop=True)
                lg2_sb = sb.tile([1, s1p], f32, tag="lg2_sb")
                nc.vector.tensor_copy(out=lg2_sb[:], in_=lg2_ps[:])
                nc.sync.dma_start(
                    out=cc_in[0:1, 0:2 * v1p].rearrange("a (v f) -> (a v) f",
                                                        v=v1p),
                    in_=h2f_part[:, 0:2])
                nc.sync.dma_start(out=cc_in[0:1, 2 * v1p:ccw], in_=lg2_sb[:])
            nc.gpsimd.collective_compute(
                "AllGather", mybir.AluOpType.bypass,
                replica_groups=[list(range(NCORES))],
                ins=[cc_in.opt()], outs=[cc_out.opt()])
            ccg8 = sb.tile([NCORES, ccw], f32, tag="ccg8")
            nc.sync.dma_start(
                out=ccg8[:],
                in_=cc_out[0:1, :].rearrange("a (r w) -> (a r) w", r=NCORES))
            red_ps = ps.tile([1, ccw], f32, tag="mm_a", name="red_ps")
            nc.tensor.matmul(out=red_ps[:], lhsT=ones_v, rhs=ccg8[:],
                             start=True, stop=True)

            # ---- layer-2 softmax at mask node (redundant on all cores) ----
            s1n, v1n = meta["s1n"], meta["v1n"]
            if packed:
                raw2 = red_ps[:].rearrange("a (p w) -> a w p", w=3)[:, 0, :]
                h2view_src = red_ps[:].rearrange(
                    "a (p w) -> a w p", w=3)[:, 1:3, 0:v1n]
            else:
                raw2 = red_ps[:, 2 * v1p:ccw]
                h2view_src = red_ps[:, 0:2 * v1p].rearrange(
                    "a (v f) -> a f v", f=2)[:, :, 0:v1n]
            al2w = P if packed else s1p
            al2t = sb.tile([1, al2w], f32, tag="al2t")
            tmp2 = sb.tile([1, al2w], f32, tag="tmp2")
            nc.vector.tensor_scalar_mul(out=tmp2[:], in0=raw2, scalar1=0.2)
            nc.vector.tensor_tensor(out=al2t[:], in0=raw2, in1=tmp2[:],
                                    op=mybir.AluOpType.max)
            al2 = al2t[:]
            nmx2 = sb.tile([1, 1], f32, tag="nmx2")
            nc.vector.reduce_max(out=nmx2[:], in_=al2[:, 0:s1n],
                                 axis=mybir.AxisListType.X, negate=True)
            nc.scalar.activation(out=al2[:, 0:s1n], in_=al2[:, 0:s1n],
                                 func=mybir.ActivationFunctionType.Exp,
                                 bias=nmx2[:, 0:1])
            sm2 = sb.tile([1, 1], f32, tag="sm2")
            nc.vector.reduce_sum(out=sm2[:], in_=al2[:, 0:s1n],
                                 axis=mybir.AxisListType.X)

            res_sb = sb.tile([1, 2], f32, tag="res_sb")
            if meta["s1_ident"]:
                # sources unique -> alpha2 aligns with V1 rows directly
                wb = al2[:, 0:v1n].rearrange(
                    "a (o v) -> a o v", o=1).to_broadcast([1, 2, v1n])
                prod = sb.tile([1, 2, v1n], f32, tag="prod")
                nc.vector.tensor_tensor(out=prod[:], in0=wb, in1=h2view_src,
                                        op=mybir.AluOpType.mult)
                nc.vector.reduce_sum(out=res_sb[:], in_=prod[:],
                                     axis=mybir.AxisListType.X)
                # normalize by the softmax denominator
                rc2 = sb.tile([1, 1], f32, tag="rc2")
                nc.vector.reciprocal(out=rc2[:], in_=sm2[:])
                nc.vector.tensor_scalar_mul(out=res_sb[:], in0=res_sb[:],
                                            scalar1=rc2[:])
            else:
                # general path: w = (GT @ alpha2^T) / denom, out = w.T @ h2f
                w_ps = ps.tile([1, v1p], f32, tag="mm_b", name="w_ps")
                for k in range(n_s1t):
                    a2T = ps.tile([P, 1], f32, tag="tp", bufs=2, name="a2T")
                    nc.tensor.transpose(out=a2T[:],
                                        in_=al2[:, k * P:(k + 1) * P],
                                        identity=ident[:1, :1])
                    a2Ts = sb.tile([P, 1], f32, tag="a2Ts")
                    nc.vector.tensor_copy(out=a2Ts[:], in_=a2T[:])
                    nc.tensor.matmul(out=w_ps[:], lhsT=a2Ts[:],
                                     rhs=gt_v[:, k, :],
                                     start=(k == 0), stop=(k == n_s1t - 1))
                rc2 = sb.tile([1, 1], f32, tag="rc2")
                nc.vector.reciprocal(out=rc2[:], in_=sm2[:])
                w_row = sb.tile([1, v1p], f32, tag="w_row")
                nc.vector.tensor_scalar_mul(out=w_row[:], in0=w_ps[:],
                                            scalar1=rc2[:])
                wb = w_row[:, 0:v1n].rearrange(
                    "a (o v) -> a o v", o=1).to_broadcast([1, 2, v1n])
                prod2 = sb.tile([1, 2, v1n], f32, tag="prod2")
                nc.vector.tensor_tensor(out=prod2[:], in0=wb, in1=h2view_src,
                                        op=mybir.AluOpType.mult)
                nc.vector.reduce_sum(out=res_sb[:], in_=prod2[:],
                                     axis=mybir.AxisListType.X)

            nc.vector.tensor_add(out=res_sb[:], in0=res_sb[:], in1=oxm_sb[:])
            nc.vector.tensor_add(out=res_sb[:], in0=res_sb[:], in1=bias3_v)
            nc.sync.dma_start(out=d_res[:], in_=res_sb[:])

    nc.compile()
    return nc


_CACHE = {}


def _get_nc(meta):
    key = repr(sorted(meta.items()))
    if key not in _CACHE:
        _CACHE[key] = _build(meta)
    return _CACHE[key]


def make_in_maps(**inputs):
    """Host preprocessing: shard/fold inputs into per-core input maps."""
    x = np.asarray(inputs["x"], np.float32)
    n_nodes = x.shape[0]
    meta, host = _preprocess(inputs["edge_index"], inputs["mask_idx"], n_nodes)

    W1 = np.asarray(inputs["W1"], np.float32)
    att_s1 = np.asarray(inputs["att_src1"], np.float32)
    att_d1 = np.asarray(inputs["att_dst1"], np.float32)
    b1 = np.asarray(inputs["b1"], np.float32)
    W2 = np.asarray(inputs["W2"], np.float32)
    att_s2 = np.asarray(inputs["att_src2"], np.float32)
    att_d2 = np.asarray(inputs["att_dst2"], np.float32)
    b2 = np.asarray(inputs["b2"], np.float32)
    fc_w = np.asarray(inputs["fc_w"], np.float32)
    fc_b = np.asarray(inputs["fc_b"], np.float32)
    cls_w = np.asarray(inputs["cls_w"], np.float32)
    cls_b = np.asarray(inputs["cls_b"], np.float32)

    Ws1 = np.einsum("chf,hf->ch", W1.reshape(C, H1, OUT), att_s1)  # [C, H1]
    Wd1 = np.einsum("chf,hf->ch", W1.reshape(C, H1, OUT), att_d1)
    Ws2 = W2 @ att_s2[0]                                           # [H1*OUT]
    Wd2 = W2 @ att_d2[0]
    # classifier fold: out = cat @ fc_w @ cls_w + (fc_b @ cls_w + cls_b)
    wf = fc_w @ cls_w                                              # [1536, 2]
    wf_top, wf_bot = wf[:OUT], wf[OUT:]
    w2fold = W2 @ wf_top                                           # [6144, 2]
    bias3 = (b2 @ wf_top + fc_b @ cls_w + cls_b).reshape(1, 2).astype(np.float32)

    n_s1t, v1p, s1p = meta["n_s1t"], meta["v1p"], meta["s1p"]
    n_et = meta["n_et"]
    s2p = n_et * P
    gt_pad = np.zeros((n_s1t * P, v1p), np.float32)
    gt_pad[:s1p] = host["gt"]

    # pre-gathered + pre-transposed x rows (index-select = sharding)
    s2p_ = meta["n_et"] * P
    xg = x[host["src_ids"]]                                        # [s2p, 768]
    xgt3 = np.ascontiguousarray(xg.T).reshape(KC, P, s2p_)
    ws13 = Ws1.reshape(KC, P, H1)
    xga = np.concatenate([xgt3, ws13], axis=2)                     # [KC,128,s2p+8]
    xga = np.ascontiguousarray(
        xga.transpose(1, 0, 2).reshape(P, KC * (s2p_ + H1)))
    xv = x[host["v1_ids"]]                                         # [v1p, 768]
    xvt = _chunked(np.ascontiguousarray(xv.T))                     # [128, KC*v1p]
    ones8 = np.ones((NCORES, 1), np.float32)

    lay, cw = _const_layout(meta)

    def fill(cst, name, arr):
        rows, off, cols = lay[name]
        assert arr.shape == (rows, cols), (name, arr.shape, (rows, cols))
        cst[0:rows, off:off + cols] = arr

    m01_pack = np.concatenate(
        [host["m01"][t * P:(t + 1) * P] for t in range(n_et)], axis=1)

    in_maps = []
    for i in range(NCORES):
        w1blk = np.ascontiguousarray(W1[:, i * OUT:(i + 1) * OUT])
        w2fblk = np.concatenate(
            [w2fold[i * OUT:(i + 1) * OUT, :],
             Ws2[i * OUT:(i + 1) * OUT, None],
             Wd2[i * OUT:(i + 1) * OUT, None]], axis=1)            # [768, 4]
        head = np.zeros((H1, 1), np.float32)
        head[i % H1, 0] = 1.0
        cst = np.zeros((P, cw), np.float32)
        fill(cst, "xvt", xvt)
        fill(cst, "ones", ones8)
        fill(cst, "wd1", _chunked(Wd1))
        fill(cst, "w2f", _chunked(w2fblk))
        fill(cst, "wfb", _chunked(np.ascontiguousarray(wf_bot)))
        fill(cst, "b1", _colmajor(b1[i * OUT:(i + 1) * OUT]))
        fill(cst, "xm", _colmajor(np.ascontiguousarray(x[meta["m"]])))
        fill(cst, "m01", m01_pack)
        fill(cst, "m01t", host["m01t"])
        fill(cst, "g", host["g"])
        fill(cst, "gm", host["gm"])
        fill(cst, "gt", _chunked(gt_pad))
        fill(cst, "padbias", host["padbias"])
        fill(cst, "bias3", bias3)
        fill(cst, "head", head)
        im = {
            "xga": xga,
            "cst": cst,
        }
        for c in range(KC):
            im[f"w1c{c}"] = np.ascontiguousarray(w1blk[c * P:(c + 1) * P, :])
        in_maps.append(im)
    return meta, in_maps


def kernel(**inputs):
    meta, in_maps = make_in_maps(**inputs)
    nc = _get_nc(meta)
    res = run_bass_kernel_spmd(nc, in_maps, core_ids=list(range(NCORES)))
    return res.results[0]["res"].astype(np.float32)


# revision 26
# speedup vs baseline: 1.0254x; 1.0038x over previous
"""Trainium2 Bass kernel for the 2-layer GAT node-classification head.

The reference reads only h2[mask_idx] and x[mask_idx] for the classifier, so
the exact computation collapses to mask_idx's 2-hop in-neighborhood:

  layer 1: h1 = x @ W1 is needed only at sources of in-edges of V1
           (V1 = sources of mask's in-edges), one row per edge in S2.
  layer 2: h2 = elu(gat1) @ W2 is needed only at rows V1, and the final
           classifier (fc -> cls, two consecutive affine maps) folds into a
           single [1536, 2] matrix on the host, so layer-2's GEMM contracts
           into W2 @ fold (4 columns: 2 logits + a_src2 + a_dst2).

Sharding over 8 cores:
  - layer-1 GEMM + attention by head (H1=8 -> head i on core i)
  - layer-2 folded GEMM by contraction block (core i contracts the head-i
    block of elu(h1)); one AllReduce(add) of the small partial
  - everything after the AllReduce is tiny and runs redundantly on all cores

Host preprocessing: gather + transpose the needed x rows (index-select is
part of sharding), fold attention vectors and the classifier into the weight
matrices, and build one-hot scatter matrices plus a uniform-stride edge
layout so segment softmax lowers to batched strided reductions.
"""

import numpy as np

import concourse.bass as bass
import concourse.mybir as mybir
import concourse.tile as tile
from concourse import bacc
from concourse.bass_utils import run_bass_kernel_spmd
from concourse.masks import make_identity

NCORES = 8
P = 128
C = 768          # input feature dim
H1 = 8           # layer-1 heads
OUT = 768        # per-head feature dim
KC = C // P      # 6 k-chunks of 128 over a 768 contraction
W2F = 4          # folded layer-2 rhs cols: [cls0 cls1 a_src2 a_dst2]
NEG = -1.0e30    # padding logit

f32 = mybir.dt.float32
f32r = mybir.dt.float32r
bf16 = mybir.dt.bfloat16
i32 = mybir.dt.int32
GEMM_DT = f32r   # single-pass fp32 matmul: full DMA bytes, 4x PE rate


# ---------------------------------------------------------------- host graph
def _preprocess(edge_index, mask_idx, n_nodes):
    """Extract the 2-hop in-neighborhood of mask_idx and pack it into
    uniform-stride group tiles. Everything in meta is compile-time python."""
    ei = np.asarray(edge_index).astype(np.int64)
    m = int(np.asarray(mask_idx))
    src_all = np.concatenate([ei[0], np.arange(n_nodes, dtype=np.int64)])
    dst_all = np.concatenate([ei[1], np.arange(n_nodes, dtype=np.int64)])

    s1_pos = np.nonzero(dst_all == m)[0]          # in-edges of m (incl self-loop)
    s1_src = src_all[s1_pos].tolist()
    v1 = list(dict.fromkeys(s1_src))              # unique sources, first-occurrence
    v1n = len(v1)
    v1p = max(v1n, 2)
    assert v1n <= P, f"in-degree of mask node too large: {v1n}"
    v1_row = {v: r for r, v in enumerate(v1)}
    s1n = len(s1_src)
    n_s1t = max(1, -(-s1n // P))
    s1p = n_s1t * P
    assert s1p <= 512, f"mask in-degree {s1n} exceeds 512"
    # layer-2 gather is the identity when every in-edge has a distinct source
    s1_ident = s1n == v1n

    # S2: in-edges of each v in V1, at uniform stride gmax within tiles
    groups = [src_all[np.nonzero(dst_all == v)[0]].tolist() for v in v1]
    gmax = max(len(g) for g in groups)
    assert gmax <= P, f"in-degree {gmax} exceeds {P}"
    gpt = P // gmax                               # groups per 128-slot tile
    n_et = -(-v1n // gpt)
    s2p = n_et * P

    src_ids = np.zeros(s2p, np.int64)             # padded with node 0
    m01 = np.zeros((s2p, v1p), np.float32)
    padbias = np.full((H1, s2p), NEG, np.float32)
    ngs = []                                      # groups in each tile
    for t in range(n_et):
        gs = groups[t * gpt:(t + 1) * gpt]
        ngs.append(len(gs))
        for j, srcs in enumerate(gs):
            v_row = t * gpt + j
            lo = t * P + j * gmax
            src_ids[lo:lo + len(srcs)] = srcs
            m01[lo:lo + len(srcs), v_row] = 1.0
            padbias[:, lo:lo + len(srcs)] = 0.0

    v1_ids = np.zeros(v1p, np.int64)
    v1_ids[:v1n] = np.array(v1, np.int64)

    g_mat = np.zeros((v1p, s1p), np.float32)      # a_src2 gather (src of S1 edge)
    gm_mat = np.zeros((v1p, s1p), np.float32)     # a_dst2 broadcast (row of m)
    for e, s in enumerate(s1_src):
        g_mat[v1_row[s], e] = 1.0
        gm_mat[v1_row[m], e] = 1.0
    gt_mat = np.ascontiguousarray(g_mat.T)        # [s1p, v1p]

    meta = dict(m=m, v1n=v1n, v1p=v1p, s1n=s1n, s1p=s1p, n_s1t=n_s1t,
                n_et=n_et, gmax=gmax, ngs=tuple(ngs), s1_ident=s1_ident)
    host = dict(src_ids=src_ids, v1_ids=v1_ids, m01=m01, padbias=padbias,
                m01t=np.ascontiguousarray(m01.T), g=g_mat, gm=gm_mat,
                gt=gt_mat)
    return meta, host


def _chunked(w):
    """[K, N] -> [128, (K//128)*N] with chunk-major free layout for one DMA."""
    k, n = w.shape
    assert k % P == 0
    return np.ascontiguousarray(
        w.reshape(k // P, P, n).transpose(1, 0, 2).reshape(P, (k // P) * n))


def _colmajor(v):
    """[768] -> [128, 6] column-chunk layout."""
    return np.ascontiguousarray(v.reshape(KC, P).T)


def _const_layout(meta):
    """Column layout of the packed-constants tensor, shared host/build."""
    v1p, s1p, n_s1t = meta["v1p"], meta["s1p"], meta["n_s1t"]
    s2p = meta["n_et"] * P
    pieces = [
        ("xvt", P, KC * v1p),
        ("ones", NCORES, 1),
        ("wd1", P, KC * H1),
        ("w2f", P, KC * W2F),
        ("wfb", P, KC * 2),
        ("b1", P, KC),
        ("xm", P, KC),
        ("m01", P, meta["n_et"] * v1p),
        ("m01t", v1p, s2p),
        ("g", v1p, s1p),
        ("gm", v1p, s1p),
        ("gt", P, n_s1t * v1p),
        ("padbias", H1, s2p),
        ("bias3", 1, 2),
        ("head", H1, 1),
    ]
    lay, off = {}, 0
    for name, rows, cols in pieces:
        lay[name] = (rows, off, cols)
        off += cols
    return lay, off


# ---------------------------------------------------------------- bass build
def _build(meta):
    v1p, s1p, n_s1t, n_et = meta["v1p"], meta["s1p"], meta["n_s1t"], meta["n_et"]
    gmax, ngs = meta["gmax"], meta["ngs"]
    s2p = n_et * P
    packed = n_s1t == 1
    ccw = P * 3 if packed else 2 * v1p + s1p      # AllGather payload
    lay, cw = _const_layout(meta)

    nc = bacc.Bacc("TRN2", target_bir_lowering=False, debug=False,
                   enable_asserts=True, num_devices=NCORES)

    d_xga = nc.dram_tensor("xga", [P, KC * (s2p + H1)], GEMM_DT,
                           kind="ExternalInput")
    d_cst = nc.dram_tensor("cst", [P, cw], f32, kind="ExternalInput")
    d_w1 = [nc.dram_tensor(f"w1c{c}", [P, OUT], GEMM_DT, kind="ExternalInput")
            for c in range(KC)]
    d_res = nc.dram_tensor("res", [1, 2], f32, kind="ExternalOutput")

    with tile.TileContext(nc) as tc:
        with (
            tc.tile_pool(name="const", bufs=1) as cpool,
            tc.tile_pool(name="sbuf", bufs=2) as sb,
            tc.tile_pool(name="big", bufs=1) as bigp,
            tc.tile_pool(name="ps", bufs=1, space="PSUM") as ps,
            tc.tile_pool(name="dram", bufs=1, space="DRAM") as dr,
        ):
            # ---- input loads: critical pieces first ----
            xga_sb = bigp.tile([P, KC, s2p + H1], GEMM_DT, tag="xga")
            nc.sync.dma_start(out=xga_sb[:], in_=d_xga[:].rearrange(
                "p (k n) -> p k n", k=KC))
            cst = cpool.tile([P, cw], f32, tag="cst")
            nc.sync.dma_start(out=cst[:], in_=d_cst[:])
            w1_sb = [bigp.tile([P, OUT], GEMM_DT, tag=f"w1_{c}", name=f"w1_{c}")
                     for c in range(KC)]
            for c in range(KC):
                nc.sync.dma_start(out=w1_sb[c][:], in_=d_w1[c][:])

            def cv(name):
                rows, off, cols = lay[name]
                return cst[0:rows, off:off + cols]

            xvt_v = cv("xvt").rearrange("p (k n) -> p k n", k=KC)
            ones_v = cv("ones")
            wd1_v = cv("wd1").rearrange("p (k n) -> p k n", k=KC)
            w2f_v = cv("w2f").rearrange("p (k n) -> p k n", k=KC)
            wfb_v = cv("wfb").rearrange("p (k n) -> p k n", k=KC)
            b1_v = cv("b1")
            xm_v = cv("xm")
            m01_v = cv("m01").rearrange("p (t n) -> p t n", t=n_et)
            m01t_v = cv("m01t")
            g_v = cv("g")
            gm_v = cv("gm")
            gt_v = cv("gt").rearrange("p (k n) -> p k n", k=n_s1t)
            pad_v = cv("padbias")
            bias3_v = cv("bias3")
            head_v = cv("head")

            ident = cpool.tile([P, P], f32, tag="ident")
            make_identity(nc, ident[:])

            # ---- attention inputs: a_src per edge, a_dst per node ----
            asT_sb = []
            for t in range(n_et):
                ap_s = ps.tile([P, H1], f32, tag="mm_b", name="ap_s")
                for c in range(KC):
                    nc.tensor.matmul(out=ap_s[:],
                                     lhsT=xga_sb[:, c, t * P:(t + 1) * P],
                                     rhs=xga_sb[:, c, s2p:s2p + H1],
                                     start=(c == 0), stop=(c == KC - 1))
                asb = sb.tile([P, H1], f32, tag=f"as_{t}", name=f"as_{t}")
                nc.vector.tensor_copy(out=asb[:], in_=ap_s[:])
                at = ps.tile([H1, P], f32, tag="tp", bufs=2, name="at")
                nc.tensor.transpose(out=at[:], in_=asb[:], identity=ident[:])
                at2 = sb.tile([H1, P], f32, tag=f"asT_{t}", name=f"asT_{t}")
                nc.vector.tensor_copy(out=at2[:], in_=at[:])
                asT_sb.append(at2)
            adv_ps = ps.tile([v1p, H1], f32, tag="mm_b", name="adv")
            for c in range(KC):
                nc.tensor.matmul(out=adv_ps[:], lhsT=xvt_v[:, c, :],
                                 rhs=wd1_v[:, c, :],
                                 start=(c == 0), stop=(c == KC - 1))
            adv_sb = sb.tile([v1p, H1], f32, tag="adv_sb")
            nc.vector.tensor_copy(out=adv_sb[:], in_=adv_ps[:])

            # ---- layer-1 logits + batched segment softmax (all heads) ----
            logit = sb.tile([H1, s2p], f32, tag="logit")
            for t in range(n_et):
                adT = ps.tile([H1, P], f32, tag="tp", bufs=2, name="adT")
                nc.tensor.matmul(out=adT[:], lhsT=adv_sb[:],
                                 rhs=m01t_v[:, t * P:(t + 1) * P],
                                 start=True, stop=True)
                nc.vector.tensor_add(out=logit[:, t * P:(t + 1) * P],
                                     in0=asT_sb[t][:], in1=adT[:])
            # leaky relu + padding mask
            tmp = sb.tile([H1, s2p], f32, tag="ltmp")
            nc.vector.tensor_scalar_mul(out=tmp[:], in0=logit[:], scalar1=0.2)
            nc.vector.tensor_tensor(out=logit[:], in0=logit[:], in1=tmp[:],
                                    op=mybir.AluOpType.max)
            nc.vector.tensor_add(out=logit[:], in0=logit[:], in1=pad_v)
            # per-group max-shift, exp, normalize (strided batched form)
            for t in range(n_et):
                ng = ngs[t]
                view = logit[:, t * P:t * P + ng * gmax].rearrange(
                    "h (g e) -> h g e", e=gmax)
                mx = sb.tile([H1, ng], f32, tag=f"mx{t}", name=f"mx{t}")
                nc.vector.reduce_max(out=mx[:], in_=view,
                                     axis=mybir.AxisListType.X)
                mxb = mx[:].rearrange("h (g o) -> h g o", o=1).to_broadcast(
                    [H1, ng, gmax])
                nc.vector.tensor_tensor(out=view, in0=view, in1=mxb,
                                        op=mybir.AluOpType.subtract)
            nc.scalar.activation(out=logit[:], in_=logit[:],
                                 func=mybir.ActivationFunctionType.Exp)
            for t in range(n_et):
                ng = ngs[t]
                view = logit[:, t * P:t * P + ng * gmax].rearrange(
                    "h (g e) -> h g e", e=gmax)
                sm = sb.tile([H1, ng], f32, tag=f"sm{t}", name=f"sm{t}")
                nc.vector.reduce_sum(out=sm[:], in_=view,
                                     axis=mybir.AxisListType.X)
                rc = sb.tile([H1, ng], f32, tag=f"rc{t}", name=f"rc{t}")
                nc.vector.reciprocal(out=rc[:], in_=sm[:])
                rcb = rc[:].rearrange("h (g o) -> h g o", o=1).to_broadcast(
                    [H1, ng, gmax])
                nc.vector.tensor_tensor(out=view, in0=view, in1=rcb,
                                        op=mybir.AluOpType.mult)
            # alpha column for this core's head + alpha-scaled selection
            a_sel = []
            for t in range(n_et):
                acol = ps.tile([P, 1], f32, tag="tp", bufs=2, name="acol")
                nc.tensor.matmul(out=acol[:],
                                 lhsT=logit[:, t * P:(t + 1) * P],
                                 rhs=head_v, start=True, stop=True)
                acs = sb.tile([P, 1], f32, tag=f"acol_sb{t}", name=f"acol_sb{t}")
                nc.vector.tensor_copy(out=acs[:], in_=acol[:])
                asel = sb.tile([P, v1p], f32, tag=f"a_sel{t}", name=f"a_sel{t}")
                nc.vector.tensor_scalar(out=asel[:], in0=m01_v[:, t, :],
                                        scalar1=acs[:], scalar2=None,
                                        op0=mybir.AluOpType.mult)
                a_sel.append(asel)

            # ---- the big per-head GEMM1: h1 = x_src @ W1_head ----
            h1_sb = []
            for t in range(n_et):
                hp_a = ps.tile([P, 512], f32, tag="mm_a", name="hp_a")
                hp_b = ps.tile([P, 256], f32, tag="mm_b", name="hp_b")
                for c in range(KC):
                    nc.tensor.matmul(out=hp_a[:],
                                     lhsT=xga_sb[:, c, t * P:(t + 1) * P],
                                     rhs=w1_sb[c][:, 0:512],
                                     start=(c == 0), stop=(c == KC - 1))
                for c in range(KC):
                    nc.tensor.matmul(out=hp_b[:],
                                     lhsT=xga_sb[:, c, t * P:(t + 1) * P],
                                     rhs=w1_sb[c][:, 512:OUT],
                                     start=(c == 0), stop=(c == KC - 1))
                h1t = sb.tile([P, OUT], f32, tag=f"h1_{t}", name=f"h1_{t}")
                nc.vector.tensor_copy(out=h1t[:, 0:512], in_=hp_a[:])
                nc.vector.tensor_copy(out=h1t[:, 512:OUT], in_=hp_b[:])
                h1_sb.append(h1t)

            # ---- xm @ Wf_bot partial (independent of the collective) ----
            oxm_ps = ps.tile([1, 2], f32, tag="oxm", name="oxm_ps")
            for c in range(KC):
                nc.tensor.matmul(out=oxm_ps[:], lhsT=xm_v[:, c:c + 1],
                                 rhs=wfb_v[:, c, :],
                                 start=(c == 0), stop=(c == KC - 1))
            oxm_sb = sb.tile([1, 2], f32, tag="oxm_sb")
            nc.vector.tensor_add(out=oxm_sb[:], in0=oxm_ps[:], in1=bias3_v)

            # ---- aggregation + bias, batched elu, folded layer-2 partial ----
            helu = sb.tile([P, KC, v1p], f32, tag="helu")
            assert KC * v1p <= 512
            agg = ps.tile([P, KC * v1p], f32, tag="agg", bufs=2, name="agg")
            for c in range(KC):
                for t in range(n_et):
                    nc.tensor.matmul(out=agg[:, c * v1p:(c + 1) * v1p],
                                     lhsT=h1_sb[t][:, c * P:(c + 1) * P],
                                     rhs=a_sel[t][:], start=(t == 0),
                                     stop=(t == n_et - 1))
            b1b = b1_v.rearrange("p (k o) -> p k o", o=1).to_broadcast(
                [P, KC, v1p])
            nc.vector.tensor_tensor(
                out=helu[:], in0=agg[:].rearrange("p (k n) -> p k n", k=KC),
                in1=b1b, op=mybir.AluOpType.add)
            # elu(x) = max(x,0) + exp(min(x,0)) - 1, one pass over all chunks
            hall = helu[:].rearrange("p k n -> p (k n)")
            mn = sb.tile([P, KC * v1p], f32, tag="mn")
            nc.vector.tensor_scalar_min(out=mn[:], in0=hall, scalar1=0.0)
            nc.scalar.activation(out=mn[:], in_=mn[:],
                                 func=mybir.ActivationFunctionType.Exp)
            nc.vector.tensor_scalar_max(out=hall, in0=hall, scalar1=0.0)
            nc.vector.tensor_add(out=hall, in0=hall, in1=mn[:])
            nc.vector.tensor_scalar_add(out=hall, in0=hall, scalar1=-1.0)
            h2f_ps = ps.tile([v1p, W2F], f32, tag="h2f", name="h2f")
            for c in range(KC):
                nc.tensor.matmul(out=h2f_ps[:], lhsT=helu[:, c, :],
                                 rhs=w2f_v[:, c, :],
                                 start=(c == 0), stop=(c == KC - 1))
            h2f_part = sb.tile([v1p, W2F], f32, tag="h2f_part")
            nc.vector.tensor_copy(out=h2f_part[:], in_=h2f_ps[:])

            # layer-2 logits are linear in h2f -> fold into the AllGather
            cc_in = dr.tile([1, ccw], f32, tag="cc_in", name="cc_in")
            cc_out = dr.tile([1, NCORES * ccw], f32, tag="cc_out",
                             name="cc_out")
            if packed:
                # transposed logits land partition-major next to h2f cols so
                # one staging tile covers the whole payload in a single DMA
                lgT_ps = ps.tile([P, 1], f32, tag="tp", bufs=2, name="lgT")
                nc.tensor.matmul(out=lgT_ps[:], lhsT=g_v[:, 0:P],
                                 rhs=h2f_part[:, 2:3], start=True, stop=False)
                nc.tensor.matmul(out=lgT_ps[:], lhsT=gm_v[:, 0:P],
                                 rhs=h2f_part[:, 3:4], start=False, stop=True)
                stg = sb.tile([P, 3], f32, tag="stg")
                nc.vector.tensor_copy(out=stg[:, 0:1], in_=lgT_ps[:])
                nc.vector.tensor_copy(out=stg[0:v1p, 1:3], in_=h2f_ps[:, 0:2])
                nc.sync.dma_start(
                    out=cc_in[0:1, :].rearrange("a (p w) -> (a p) w", p=P),
                    in_=stg[:])
            else:
                lg2_ps = ps.tile([1, s1p], f32, tag="mm_a", name="lg2")
                nc.tensor.matmul(out=lg2_ps[:], lhsT=h2f_part[:, 2:3],
                                 rhs=g_v, start=True, stop=False)
                nc.tensor.matmul(out=lg2_ps[:], lhsT=h2f_part[:, 3:4],
                                 rhs=gm_v, start=False, stop=True)
                lg2_sb = sb.tile([1, s1p], f32, tag="lg2_sb")
                nc.vector.tensor_copy(out=lg2_sb[:], in_=lg2_ps[:])
                nc.sync.dma_start(
                    out=cc_in[0:1, 0:2 * v1p].rearrange("a (v f) -> (a v) f",
                                                        v=v1p),
                    in_=h2f_part[:, 0:2])
                nc.sync.dma_start(out=cc_in[0:1, 2 * v1p:ccw], in_=lg2_sb[:])
            nc.gpsimd.collective_compute(
                "AllGather", mybir.AluOpType.bypass,
                replica_groups=[list(range(NCORES))],
                ins=[cc_in.opt()], outs=[cc_out.opt()])
            ccg8 = sb.tile([NCORES, ccw], f32, tag="ccg8")
            nc.sync.dma_start(
                out=ccg8[:],
                in_=cc_out[0:1, :].rearrange("a (r w) -> (a r) w", r=NCORES))
            red_ps = ps.tile([1, ccw], f32, tag="mm_a", name="red_ps")
            nc.tensor.matmul(out=red_ps[:], lhsT=ones_v, rhs=ccg8[:],
                             start=True, stop=True)

            # ---- layer-2 softmax at mask node (redundant on all cores) ----
            s1n, v1n = meta["s1n"], meta["v1n"]
            if packed:
                raw2 = red_ps[:].rearrange("a (p w) -> a w p", w=3)[:, 0, :]
                h2view_src = red_ps[:].rearrange(
                    "a (p w) -> a w p", w=3)[:, 1:3, 0:v1n]
            else:
                raw2 = red_ps[:, 2 * v1p:ccw]
                h2view_src = red_ps[:, 0:2 * v1p].rearrange(
                    "a (v f) -> a f v", f=2)[:, :, 0:v1n]
            al2w = P if packed else s1p
            al2t = sb.tile([1, al2w], f32, tag="al2t")
            tmp2 = sb.tile([1, al2w], f32, tag="tmp2")
            nc.vector.tensor_scalar_mul(out=tmp2[:], in0=raw2, scalar1=0.2)
            nc.vector.tensor_tensor(out=al2t[:], in0=raw2, in1=tmp2[:],
                                    op=mybir.AluOpType.max)
            al2 = al2t[:]
            nmx2 = sb.tile([1, 1], f32, tag="nmx2")
            nc.vector.reduce_max(out=nmx2[:], in_=al2[:, 0:s1n],
                                 axis=mybir.AxisListType.X, negate=True)
            nc.scalar.activation(out=al2[:, 0:s1n], in_=al2[:, 0:s1n],
                                 func=mybir.ActivationFunctionType.Exp,
                                 bias=nmx2[:, 0:1])
            sm2 = sb.tile([1, 1], f32, tag="sm2")
            nc.vector.reduce_sum(out=sm2[:], in_=al2[:, 0:s1n],
                                 axis=mybir.AxisListType.X)

            res_sb = sb.tile([1, 2], f32, tag="res_sb")
            if meta["s1_ident"]:
                # sources unique -> alpha2 aligns with V1 rows directly
                wb = al2[:, 0:v1n].rearrange(
                    "a (o v) -> a o v", o=1).to_broadcast([1, 2, v1n])
                prod = sb.tile([1, 2, v1n], f32, tag="prod")
                nc.vector.tensor_tensor(out=prod[:], in0=wb, in1=h2view_src,
                                        op=mybir.AluOpType.mult)
                nc.vector.reduce_sum(out=res_sb[:], in_=prod[:],
                                     axis=mybir.AxisListType.X)
                # normalize by the softmax denominator
                rc2 = sb.tile([1, 1], f32, tag="rc2")
                nc.vector.reciprocal(out=rc2[:], in_=sm2[:])
                nc.vector.tensor_scalar_mul(out=res_sb[:], in0=res_sb[:],
                                            scalar1=rc2[:])
            else:
                # general path: w = (GT @ alpha2^T) / denom, out = w.T @ h2f
                w_ps = ps.tile([1, v1p], f32, tag="mm_b", name="w_ps")
                for k in range(n_s1t):
                    a2T = ps.tile([P, 1], f32, tag="tp", bufs=2, name="a2T")
                    nc.tensor.transpose(out=a2T[:],
                                        in_=al2[:, k * P:(k + 1) * P],
                                        identity=ident[:1, :1])
                    a2Ts = sb.tile([P, 1], f32, tag="a2Ts")
                    nc.vector.tensor_copy(out=a2Ts[:], in_=a2T[:])
                    nc.tensor.matmul(out=w_ps[:], lhsT=a2Ts[:],
                                     rhs=gt_v[:, k, :],
                                     start=(k == 0), stop=(k == n_s1t - 1))
                rc2 = sb.tile([1, 1], f32, tag="rc2")
                nc.vector.reciprocal(out=rc2[:], in_=sm2[:])
                w_row = sb.tile([1, v1p], f32, tag="w_row")
                nc.vector.tensor_scalar_mul(out=w_row[:], in0=w_ps[:],
                                            scalar1=rc2[:])
                wb = w_row[:, 0:v1n].rearrange(
                    "a (o v) -> a o v", o=1).to_broadcast([1, 2, v1n])
                prod2 = sb.tile([1, 2, v1n], f32, tag="prod2")
                nc.vector.tensor_tensor(out=prod2[:], in0=wb, in1=h2view_src,
                                        op=mybir.AluOpType.mult)
                nc.vector.reduce_sum(out=res_sb[:], in_=prod2[:],
                                     axis=mybir.AxisListType.X)

            nc.vector.tensor_add(out=res_sb[:], in0=res_sb[:], in1=oxm_sb[:])
            nc.sync.dma_start(out=d_res[:], in_=res_sb[:])

    nc.compile()
    return nc


_CACHE = {}


def _get_nc(meta):
    key = repr(sorted(meta.items()))
    if key not in _CACHE:
        _CACHE[key] = _build(meta)
    return _CACHE[key]


def make_in_maps(**inputs):
    """Host preprocessing: shard/fold inputs into per-core input maps."""
    x = np.asarray(inputs["x"], np.float32)
    n_nodes = x.shape[0]
    meta, host = _preprocess(inputs["edge_index"], inputs["mask_idx"], n_nodes)

    W1 = np.asarray(inputs["W1"], np.float32)
    att_s1 = np.asarray(inputs["att_src1"], np.float32)
    att_d1 = np.asarray(inputs["att_dst1"], np.float32)
    b1 = np.asarray(inputs["b1"], np.float32)
    W2 = np.asarray(inputs["W2"], np.float32)
    att_s2 = np.asarray(inputs["att_src2"], np.float32)
    att_d2 = np.asarray(inputs["att_dst2"], np.float32)
    b2 = np.asarray(inputs["b2"], np.float32)
    fc_w = np.asarray(inputs["fc_w"], np.float32)
    fc_b = np.asarray(inputs["fc_b"], np.float32)
    cls_w = np.asarray(inputs["cls_w"], np.float32)
    cls_b = np.asarray(inputs["cls_b"], np.float32)

    Ws1 = np.einsum("chf,hf->ch", W1.reshape(C, H1, OUT), att_s1)  # [C, H1]
    Wd1 = np.einsum("chf,hf->ch", W1.reshape(C, H1, OUT), att_d1)
    Ws2 = W2 @ att_s2[0]                                           # [H1*OUT]
    Wd2 = W2 @ att_d2[0]
    # classifier fold: out = cat @ fc_w @ cls_w + (fc_b @ cls_w + cls_b)
    wf = fc_w @ cls_w                                              # [1536, 2]
    wf_top, wf_bot = wf[:OUT], wf[OUT:]
    w2fold = W2 @ wf_top                                           # [6144, 2]
    bias3 = (b2 @ wf_top + fc_b @ cls_w + cls_b).reshape(1, 2).astype(np.float32)

    n_s1t, v1p, s1p = meta["n_s1t"], meta["v1p"], meta["s1p"]
    n_et = meta["n_et"]
    s2p = n_et * P
    gt_pad = np.zeros((n_s1t * P, v1p), np.float32)
    gt_pad[:s1p] = host["gt"]

    # pre-gathered + pre-transposed x rows (index-select = sharding)
    s2p_ = meta["n_et"] * P
    xg = x[host["src_ids"]]                                        # [s2p, 768]
    xgt3 = np.ascontiguousarray(xg.T).reshape(KC, P, s2p_)
    ws13 = Ws1.reshape(KC, P, H1)
    xga = np.concatenate([xgt3, ws13], axis=2)                     # [KC,128,s2p+8]
    xga = np.ascontiguousarray(
        xga.transpose(1, 0, 2).reshape(P, KC * (s2p_ + H1)))
    xv = x[host["v1_ids"]]                                         # [v1p, 768]
    xvt = _chunked(np.ascontiguousarray(xv.T))                     # [128, KC*v1p]
    ones8 = np.ones((NCORES, 1), np.float32)

    lay, cw = _const_layout(meta)

    def fill(cst, name, arr):
        rows, off, cols = lay[name]
        assert arr.shape == (rows, cols), (name, arr.shape, (rows, cols))
        cst[0:rows, off:off + cols] = arr

    m01_pack = np.concatenate(
        [host["m01"][t * P:(t + 1) * P] for t in range(n_et)], axis=1)

    in_maps = []
    for i in range(NCORES):
        w1blk = np.ascontiguousarray(W1[:, i * OUT:(i + 1) * OUT])
        w2fblk = np.concatenate(
            [w2fold[i * OUT:(i + 1) * OUT, :],
             Ws2[i * OUT:(i + 1) * OUT, None],
             Wd2[i * OUT:(i + 1) * OUT, None]], axis=1)            # [768, 4]
        head = np.zeros((H1, 1), np.float32)
        head[i % H1, 0] = 1.0
        cst = np.zeros((P, cw), np.float32)
        fill(cst, "xvt", xvt)
        fill(cst, "ones", ones8)
        fill(cst, "wd1", _chunked(Wd1))
        fill(cst, "w2f", _chunked(w2fblk))
        fill(cst, "wfb", _chunked(np.ascontiguousarray(wf_bot)))
        fill(cst, "b1", _colmajor(b1[i * OUT:(i + 1) * OUT]))
        fill(cst, "xm", _colmajor(np.ascontiguousarray(x[meta["m"]])))
        fill(cst, "m01", m01_pack)
        fill(cst, "m01t", host["m01t"])
        fill(cst, "g", host["g"])
        fill(cst, "gm", host["gm"])
        fill(cst, "gt", _chunked(gt_pad))
        fill(cst, "padbias", host["padbias"])
        fill(cst, "bias3", bias3)
        fill(cst, "head", head)
        im = {
            "xga": xga,
            "cst": cst,
        }
        for c in range(KC):
            im[f"w1c{c}"] = np.ascontiguousarray(w1blk[c * P:(c + 1) * P, :])
        in_maps.append(im)
    return meta, in_maps


def kernel(**inputs):
    meta, in_maps = make_in_maps(**inputs)
    nc = _get_nc(meta)
    res = run_bass_kernel_spmd(nc, in_maps, core_ids=list(range(NCORES)))
    return res.results[0]["res"].astype(np.float32)
